# revision 95
# baseline (speedup 1.0000x reference)
"""CAM_Module (channel attention) Trainium2 Bass kernel.

x: (16, 512, 64, 64) f32, gamma: (1,) f32
  xf = x.reshape(B, C, N)           N = 4096
  energy = xf @ xf^T                (B, C, C)
  att = softmax(max(energy) - energy, axis=-1)   == softmax(-energy) (shift-invariant)
  out = gamma * (att @ xf) + x

Sharding: data-parallel over batch, 2 batches per core on 8 cores.

Per-core pipeline (2 batches, software-pipelined across For_i reps):
  - input DMA on the SP HWDGE queue, front-loaded (b0 then b1) and
    re-issued per iteration right after the residual reads release each
    buffer, so the 33.6 MB/core HBM stream runs near peak
  - PE transpose-mode (f32 identity): xf^T k-pair chunks -> PSUM ->
    DVE/ACT copy-cast -> SBUF fp8e4m3 pair tiles [128, 2, C]
  - MM1 in fp8 DoubleRow (K=256/instr, 0.5 cycles/row): energy upper-tri
    blocks accumulated over 16 k-pairs into 4 PSUM banks; lower blocks
    mirrored via PE transpose (fp8 quantization only perturbs softmax
    logits; output error stays tiny via the exact f32 residual path)
  - softmax: DVE row-min, ACT exp(min - e) -> UNSCALED att (bf16) with
    row-sum accumulation; the gamma/Z scale is deferred to the output
  - PE transpose att -> att^T (bf16 identity) -> ACT cast to fp8 pairs
  - MM2 in fp8 DoubleRow over 2 j-pair steps; Pool builds the fp8 xf
    pair tiles (SBUF->SBUF casts; GPSIMD cannot touch PSUM)
  - DVE fuses the output move: o = po * (gamma/Z) + x (exact f32 x)
  - store on the SP HWDGE queue; batch 0's output chunks interleave
    with batch 1's transpose+MM1 stream group-by-group on the PE
"""

import sys

if "/opt/trn_rl_repo" not in sys.path:
    sys.path.insert(0, "/opt/trn_rl_repo")

from contextlib import ExitStack

import numpy as np

import concourse.bass as bass
import concourse.tile as tile
from concourse import bacc, mybir
from concourse.bass_utils import run_bass_kernel_spmd
from concourse.masks import make_identity

N_CORES = 8
B, C, H, W = 16, 512, 64, 64
N = H * W                    # 4096
BPC = B // N_CORES           # batches per core = 2
CT = C // 128                # 4 c-tiles
KT = N // 128                # 32 k-chunks (transposed layout)
KP = KT // 2                 # 16 k-pairs for DoubleRow MM1
NCH = N // 512               # 8 moving chunks for MM2

F32 = mybir.dt.float32
F32R = mybir.dt.float32r
BF16 = mybir.dt.bfloat16
FP8 = mybir.dt.float8e4
DR = mybir.MatmulPerfMode.DoubleRow


def _build_nc(reps=1, upto="full", unroll=1):
    nc = bacc.Bacc("TRN2", target_bir_lowering=False, debug=False,
                   num_devices=N_CORES)
    x_d = nc.dram_tensor("x", [BPC, C, N], F32, kind="ExternalInput").ap()
    g_d = nc.dram_tensor("gamma", [1], F32, kind="ExternalInput").ap()
    o_d = nc.dram_tensor("out", [BPC, C, N], F32, kind="ExternalOutput").ap()

    with tile.TileContext(nc) as tc, ExitStack() as ctx:
        xf_pool = ctx.enter_context(tc.tile_pool(name="xf", bufs=BPC * CT))
        xT8_pool = ctx.enter_context(tc.tile_pool(name="xT8", bufs=8))
        xf8_pool = ctx.enter_context(tc.tile_pool(name="xf8", bufs=4))
        s_pool = ctx.enter_context(tc.tile_pool(name="s", bufs=2))
        att_pool = ctx.enter_context(tc.tile_pool(name="att", bufs=CT))
        attT_pool = ctx.enter_context(tc.tile_pool(name="attT", bufs=CT))
        out_pool = ctx.enter_context(tc.tile_pool(name="outp", bufs=3))
        stat_pool = ctx.enter_context(tc.tile_pool(name="stat", bufs=4 * CT))
        one_pool = ctx.enter_context(tc.tile_pool(name="one", bufs=1))
        pT = ctx.enter_context(tc.tile_pool(name="pT", bufs=2, space="PSUM"))
        pE = ctx.enter_context(tc.tile_pool(name="pE", bufs=CT, space="PSUM"))
        pO = ctx.enter_context(tc.tile_pool(name="pO", bufs=2, space="PSUM"))

        # identities for PE transpose-mode.  The hw compiler requires
        # 32-bit matmul operands to match dtypes exactly (and f32r data
        # must come from a rounding producer), so f32 data transposes
        # with the f32 identity (2.0 cycles/row); bf16 data (att) uses a
        # bf16 identity (1.0 cycles/row).
        ident_f = one_pool.tile([128, 128], F32, tag="idf")
        make_identity(nc, ident_f[:])
        ident16 = one_pool.tile([128, 128], BF16, tag="idr")
        nc.vector.tensor_copy(ident16[:], ident_f[:])

        # broadcast gamma to all 128 partitions via K=1 matmul with ones
        g_sb = one_pool.tile([1, 1], F32, tag="gsb")
        nc.sync.dma_start(g_sb[:], g_d.rearrange("(a b) -> a b", a=1))
        ones = one_pool.tile([1, 128], F32, tag="ones")
        nc.vector.memset(ones[:], 1.0)
        pG = pT.tile([128, 1], F32, tag="pt", name="pG")
        nc.tensor.matmul(pG[:], ones[:], g_sb[:], start=True, stop=True)
        g_bc = one_pool.tile([128, 1], F32, tag="gbc")
        nc.vector.tensor_copy(g_bc[:], pG[:])

        # per-c-tile load chunks: a small first chunk so the pipeline
        # starts early, bigger ones later (amortize per-DMA fixed cost)
        CHUNKS = [(0, 512), (512, 512), (1024, 1024), (2048, 1024),
                  (3072, 1024)]

        def chunk_of(col):
            for i, (off, w) in enumerate(CHUNKS):
                if off <= col < off + w:
                    return i, col - off
            raise AssertionError(col)

        st = [dict() for _ in range(BPC)]

        def _load_tile(b, ct, q):
            off, w = CHUNKS[q]
            t = xf_pool.tile([128, w], F32, tag=f"xf{q}",
                             name=f"xf_{b}_{ct}_{q}")
            nc.sync.dma_start(
                t[:], x_d[b, ct * 128:(ct + 1) * 128, off:off + w])
            return t

        def emit_loads(b):
            # prologue loads, q-major so the first k-chunks of all four
            # c-tiles arrive first (paces the cold-start transpose stream)
            grid = [[None] * len(CHUNKS) for _ in range(CT)]
            for q in range(len(CHUNKS)):
                for ct in range(CT):
                    grid[ct][q] = _load_tile(b, ct, q)
            return grid

        def emit_loads_ct(b, ct):
            # steady-state loads for one c-tile, emitted as soon as the
            # previous iteration's residual reads of that tile are done
            # (per-tag ring order matches emit_loads: ct ascending)
            return [_load_tile(b, ct, q) for q in range(len(CHUNKS))]

        def xf_slice(b, ct, col, width):
            q, o = chunk_of(col)
            return st[b]["xf"][ct][q][:, o:o + width]

        def emit_xf8(b, engines=("act",)):
            # fp8 pair tiles for DoubleRow MM2: xf8[jP][p, i2, n] =
            # xf[256*jP + 128*i2 + p, n].  Cast chunk-by-chunk so each
            # cast can start as soon as its load chunk lands; the piece
            # stream cycles over `engines` so no single engine paces it.
            s = st[b]
            s["xf8"] = []
            i = 0
            for jp in range(CT // 2):
                t8 = xf8_pool.tile([128, 2, N], FP8, tag="xf8",
                                   name=f"xf8_{b}_{jp}")
                for i2 in range(2):
                    ct = 2 * jp + i2
                    for q, (off, w) in enumerate(CHUNKS):
                        dst = t8[:, i2, off:off + w]
                        src = s["xf"][ct][q][:]
                        eng = engines[i % len(engines)]
                        i += 1
                        if eng == "act":
                            nc.scalar.copy(dst, src)
                        elif eng == "dve":
                            nc.vector.tensor_copy(dst, src)
                        else:
                            nc.gpsimd.tensor_copy(dst, src)
                s["xf8"].append(t8)

        def emit_tr(b, kp, copy_engines=("pool", "pool")):
            # transpose k-chunks 2*kp, 2*kp+1 of all 4 c-tiles, then
            # copy-cast PSUM f32 -> fp8 pair tile [128, 2, C] (the layout
            # DoubleRow wants: contraction index n = 256*kp + 128*i2 + p)
            x8 = xT8_pool.tile([128, 2, C], FP8, tag="x8",
                               name=f"x8_{b}_{kp}")
            for i2 in range(2):
                k = 2 * kp + i2
                tp = pT.tile([128, C], F32, tag="pt", name=f"tp_{b}_{k}")
                for ct in range(CT):
                    nc.tensor.transpose(
                        tp[:, ct * 128:(ct + 1) * 128],
                        xf_slice(b, ct, k * 128, 128),
                        ident_f[:],
                    )
                # PSUM source: only DVE and ACT may read PSUM
                eng = copy_engines[i2 % len(copy_engines)]
                if eng == "act":
                    nc.scalar.copy(x8[:, i2, :], tp[:])
                else:
                    nc.vector.tensor_copy(x8[:, i2, :], tp[:])
            return x8

        # j-range computed directly per i-tile: j >= i blocks, except
        # it=3 starts at 256 (not 384): f32r matmuls with moving dim < 256
        # run at 1/4 rate, so the 256-wide block costs the same as the
        # penalized 128-wide one and computes block (3,2) for free.
        MM1_J0 = [0, 128, 256, 256]

        def emit_mm1(b, kp, x8):
            # fp8 DoubleRow: both operands [128, 2, free], K=256/instr
            for it in range(CT):
                j0 = MM1_J0[it]
                nc.tensor.matmul(
                    st[b]["e"][it][:, j0:C],
                    x8[:, :, it * 128:(it + 1) * 128],
                    x8[:, :, j0:C],
                    start=(kp == 0),
                    stop=(kp == KP - 1),
                    perf_mode=DR,
                )

        def gen_trmm1(b, kp_from=0, prefix=(), copy_engines=("pool",)):
            # generator: yields after each kp group so the caller can
            # interleave other PE work between groups
            s = st[b]
            s["e"] = [
                pE.tile([128, C], F32, tag="pe", name=f"pe_{b}_{i}")
                for i in range(CT)
            ]
            pending = list(prefix)
            for kp in range(kp_from, KP):
                pending.append(emit_tr(b, kp, copy_engines))
                if len(pending) > 1:
                    emit_mm1(b, kp - len(pending) + 1, pending.pop(0))
                yield
            base = KP - len(pending)
            for i, x8 in enumerate(pending):
                emit_mm1(b, base + i, x8)

        def emit_trmm1(b, kp_from=0, prefix=(), copy_engines=("pool",)):
            for _ in gen_trmm1(b, kp_from, prefix, copy_engines):
                pass

        def emit_mirror(b):
            # mirror lower-triangle blocks e[t][:, u] = e[u][:, t].T via
            # sbuf bounce + transpose into a scratch psum bank + DVE
            # write-back (PE never touches accumulation-grouped banks)
            e_ps = st[b]["e"]
            for t in range(1, CT):
                nmir = MM1_J0[t] // 128
                mp = pT.tile([128, C], F32, tag="pt", name=f"mp_{b}_{t}")
                for u in range(nmir):
                    mtmp = s_pool.tile([128, 128], F32, tag="mir",
                                       name=f"mir_{b}_{t}_{u}")
                    nc.vector.tensor_copy(
                        mtmp[:], e_ps[u][:, t * 128:(t + 1) * 128])
                    nc.tensor.transpose(
                        mp[:, u * 128:(u + 1) * 128], mtmp[:], ident_f[:])
                nc.vector.tensor_copy(
                    e_ps[t][:, 0:nmir * 128], mp[:, 0:nmir * 128])

        def emit_softmax(b):
            # att is kept UNSCALED (att = exp(min - e), bf16): the
            # gamma/Z row scale is deferred into the output move as
            # o = po*g + x, so recip/mul are off the softmax->MM2
            # critical chain entirely
            s = st[b]
            s["att"] = []
            s["g"] = []
            for it in range(CT):
                m = stat_pool.tile([128, 1], F32, tag="m",
                                   name=f"m_{b}_{it}")
                nc.vector.tensor_reduce(
                    m[:], s["e"][it][:], axis=mybir.AxisListType.X,
                    op=mybir.AluOpType.min,
                )
                sx = att_pool.tile([128, C], BF16, tag="a",
                                   name=f"s_{b}_{it}")
                z = stat_pool.tile([128, 1], F32, tag="z",
                                   name=f"z_{b}_{it}")
                nc.scalar.activation(
                    sx[:], s["e"][it][:], mybir.ActivationFunctionType.Exp,
                    bias=m[:], scale=-1.0, accum_out=z[:],
                )
                rz = stat_pool.tile([128, 1], F32, tag="rz",
                                    name=f"rz_{b}_{it}")
                nc.vector.reciprocal(rz[:], z[:])
                g = stat_pool.tile([128, 1], F32, tag="g",
                                   name=f"g_{b}_{it}")
                nc.vector.tensor_mul(g[:], rz[:], g_bc[:])
                s["att"].append(sx)
                s["g"].append(g)

        def emit_attT(b):
            # fp8 pair tiles for DoubleRow MM2: attT8[jP][p, i2, i] =
            # att[i, 256*jP + 128*i2 + p]
            s = st[b]
            s["attT8"] = []
            for jp in range(CT // 2):
                a8 = attT_pool.tile([128, 2, C], FP8, tag="aT",
                                    name=f"aT_{b}_{jp}")
                for i2 in range(2):
                    jt = 2 * jp + i2
                    tp = pT.tile([128, C], BF16, tag="pt",
                                 name=f"at_{b}_{jt}")
                    for it in range(CT):
                        nc.tensor.transpose(
                            tp[:, it * 128:(it + 1) * 128],
                            s["att"][it][:, jt * 128:(jt + 1) * 128],
                            ident16[:],
                        )
                    # PSUM source: GPSIMD can't access PSUM, use ACT
                    nc.scalar.copy(a8[:, i2, :], tp[:])
                s["attT8"].append(a8)

        def emit_mm2(b, its=range(CT), add_engines=("dve",),
                     borrow_pe=False, after_it=None):
            for _ in gen_mm2(b, its, add_engines, borrow_pe, after_it):
                pass

        def gen_mm2(b, its=range(CT), add_engines=("dve",),
                    borrow_pe=False, after_it=None):
            # The +x residual is fused into the PSUM->SBUF move as a
            # tensor_add of the exact f32 x, spread across `add_engines`.
            # borrow_pe: rotate po through the freed energy banks too so
            # 2+4 buffers cover the matmul->add->store chain latency.
            s = st[b]
            for it in its:
                for nch in range(NCH):
                    o = out_pool.tile([128, 512], F32, tag="o", bufs=6,
                                      name=f"o_{b}_{it}_{nch}")[:]
                    if borrow_pe and nch % 2 == 0:
                        po = pE.tile([128, C], F32, tag="pe",
                                     name=f"po_{b}_{it}_{nch}")
                    else:
                        po = pO.tile([128, 512], F32, tag="po",
                                     name=f"po_{b}_{it}_{nch}")
                    for jp in range(CT // 2):
                        nc.tensor.matmul(
                            po[:, 0:512],
                            s["attT8"][jp][:, :,
                                           it * 128:(it + 1) * 128],
                            s["xf8"][jp][:, :,
                                         nch * 512:(nch + 1) * 512],
                            start=(jp == 0),
                            stop=(jp == CT // 2 - 1),
                            perf_mode=DR,
                        )
                    # po is PSUM: only DVE can do the fused scale+add
                    nc.vector.scalar_tensor_tensor(
                        o, po[:, 0:512], s["g"][it][:],
                        xf_slice(b, it, nch * 512, 512),
                        op0=mybir.AluOpType.mult,
                        op1=mybir.AluOpType.add,
                    )
                    nc.sync.dma_start(
                        o_d[b, it * 128:(it + 1) * 128,
                            nch * 512:(nch + 1) * 512],
                        o,
                    )
                    if nch % 2 == 1:
                        yield
                if after_it is not None:
                    after_it(it)

        # Emission = per-engine execution order.  All loads go first so
        # the input HBM stream runs at full rate from t=0 (b0 before b1:
        # b0's energy needs all of b0 first).  Batch 0's MM2 and batch
        # 1's transpose+MM1 stream are interleaved group-by-group, and
        # b1's mirror+softmax are emitted before the tail of b0's MM2 so
        # they get ahead of the remaining residual adds in the in-order
        # DVE queue.  Copy-engine choices keep each phase's PSUM->SBUF
        # cast stream off whichever engine is already pacing that phase.
        def emit_iteration(loads, last=True):
            PFX = 3
            st[0]["xf"], st[1]["xf"] = loads
            next0 = [None] * CT
            next1 = [None] * CT

            def reload_cb(nxt, b):
                # next iteration's loads for c-tile `it`, emitted right
                # after this iteration's last residual read of that tile
                if last:
                    return None

                def cb(it):
                    nxt[it] = emit_loads_ct(b, it)
                return cb

            emit_xf8(0, engines=("act", "pool"))
            emit_trmm1(0, copy_engines=("dve", "dve"))
            emit_mirror(0)
            emit_softmax(0)
            # prefix transposes fill the PE bubble under b0's softmax;
            # their copies go to ACT (queued after b0's exps)
            prefix = [emit_tr(1, kp, ("act", "act")) for kp in range(PFX)]
            emit_attT(0)
            emit_xf8(1, engines=("act", "pool"))
            g1 = gen_trmm1(1, kp_from=PFX, prefix=prefix,
                           copy_engines=("act", "act"))
            g0 = gen_mm2(0)
            done0 = done1 = False
            # b0 output chunks first (they feed the store stream), one b1
            # k-pair group interposed per chunk (arrival-paced anyway)
            while not (done0 and done1):
                if not done0:
                    done0 = next(g0, "end") == "end"
                if not done1:
                    done1 = next(g1, "end") == "end"
                if done0 and not done1:
                    # b0 drained: finish b1 then its softmax
                    while not done1:
                        done1 = next(g1, "end") == "end"
            if not last:
                # next iteration's b0 loads: queued after b0's stores,
                # before b1's, and b0's buffers free right about now
                next0 = emit_loads(0)
            emit_mirror(1)
            emit_softmax(1)
            emit_attT(1)
            emit_mm2(1, borrow_pe=True)
            if not last:
                next1 = emit_loads(1)
            return next0, next1

        if reps == 1:
            for u in range(unroll):
                loads = (emit_loads(0), emit_loads(1))
                emit_iteration(loads, last=True)
        else:
            # self-contained iterations (loads at the top; the tile
            # scheduler rejects tail-load software pipelines across the
            # loop-back edge with a deadlock).  Cross-iteration overlap
            # still happens through the buffer-ring semaphores; two
            # iterations per hardware-loop body halve any per-body
            # loop-control overhead.
            def one_iter():
                ld = (emit_loads(0), emit_loads(1))
                emit_iteration(ld, last=True)

            UNR = 2 if reps >= 4 else 1
            for _ in range(reps % UNR):
                one_iter()
            with tc.For_i(0, reps // UNR, 1):
                for _ in range(UNR):
                    one_iter()

    nc.compile()
    return nc


_RUNNER = None


def _build_runner(nc=None):
    """Compile once; return a callable (xf_full, gamma) -> out_full.

    Mirrors concourse.bass2jax.run_bass_via_pjrt but caches the jitted
    shard_map executable so repeated kernel() calls don't re-lower, and
    keeps the output-seed zero buffers resident on device.
    """
    import jax
    from jax.sharding import Mesh, NamedSharding, PartitionSpec
    from jax.experimental.shard_map import shard_map

    from concourse import bass2jax, mybir as _mybir
    from concourse.bass2jax import _bass_exec_p, partition_id_tensor

    if nc is None:
        nc = _build_nc()
    bass2jax.install_neuronx_cc_hook()

    partition_name = (
        nc.partition_id_tensor.name if nc.partition_id_tensor else None
    )
    in_names, out_names, out_avals, zero_shapes = [], [], [], []
    for alloc in nc.m.functions[0].allocations:
        if not isinstance(alloc, _mybir.MemoryLocationSet):
            continue
        name = alloc.memorylocations[0].name
        if alloc.kind == "ExternalInput":
            if name != partition_name:
                in_names.append(name)
        elif alloc.kind == "ExternalOutput":
            shape = tuple(alloc.tensor_shape)
            dtype = _mybir.dt.np(alloc.dtype)
            out_names.append(name)
            out_avals.append(jax.core.ShapedArray(shape, dtype))
            zero_shapes.append((shape, dtype))
    n_params = len(in_names)
    all_names = list(in_names) + list(out_names)
    if partition_name is not None:
        all_names.append(partition_name)
    donate = tuple(range(n_params, n_params + len(out_names)))

    def _body(*args):
        operands = list(args)
        if partition_name is not None:
            operands.append(partition_id_tensor())
        return tuple(
            _bass_exec_p.bind(
                *operands,
                out_avals=tuple(out_avals),
                in_names=tuple(all_names),
                out_names=tuple(out_names),
                lowering_input_output_aliases=(),
                sim_require_finite=True,
                sim_require_nnan=True,
                nc=nc,
            )
        )

    devices = jax.devices()[:N_CORES]
    mesh = Mesh(np.asarray(devices), ("core",))
    n_in = n_params + len(out_names)
    sharded = jax.jit(
        shard_map(
            _body,
            mesh=mesh,
            in_specs=(PartitionSpec("core"),) * n_in,
            out_specs=(PartitionSpec("core"),) * len(out_names),
            check_rep=False,
        ),
        keep_unused=True,
    )

    # in_names order is discovered from allocations; map our two inputs
    assert set(in_names) == {"x", "gamma"}, in_names

    # output-seed buffers created on device once (kernel writes out fully)
    sh = NamedSharding(mesh, PartitionSpec("core"))
    zeros_dev = [
        jax.jit(
            lambda s=s, d=d: jax.numpy.zeros((N_CORES * s[0],) + s[1:], d),
            out_shardings=sh,
        )()
        for s, d in zero_shapes
    ]
    jax.block_until_ready(zeros_dev)

    def run(xf_full, gamma):
        per_in = {
            "x": xf_full,  # (16, 512, 4096) == concat of per-core (2, 512, 4096)
            "gamma": np.ascontiguousarray(
                np.broadcast_to(np.asarray(gamma, np.float32).reshape(1),
                                (N_CORES,))
            ),
        }
        concat_in = [per_in[name] for name in in_names]
        out_arrs = sharded(*concat_in, *zeros_dev)
        return np.asarray(out_arrs[out_names.index("out")])

    run.sharded = sharded
    run.zeros_dev = zeros_dev
    run.in_names = in_names
    run.out_names = out_names
    run.mesh = mesh
    return run


def _get_runner():
    global _RUNNER
    if _RUNNER is None:
        _RUNNER = _build_runner()
    return _RUNNER


def kernel(x, gamma):
    assert x.shape == (B, C, H, W)
    run = _get_runner()
    xf = np.ascontiguousarray(np.asarray(x, np.float32).reshape(B, C, N))
    g = np.asarray(gamma, np.float32)
    out = run(xf, g)
    return out.reshape(B, C, H, W).astype(np.float32, copy=False)

